# revision 26
# baseline (speedup 1.0000x reference)
"""Trainium2 Bass kernel for nn_LocalAttention (5x5 local window attention).

Contract: kernel(**inputs) takes the FULL inputs from setup_inputs() and
returns the FULL output.  Internally shards across 8 NeuronCores as
(batch b in 0..3) x (head-group hg in 0..1, 4 heads each).  Each core
computes a partial output projection; the host sums the two partials per
batch and adds b_out.

Two phases (no barrier; Tile dependency tracking orders them):
  B: qkv projection of 512-px blocks into persistent qT/kT (d-major,
     fp16) and v (pixel-major fp16 with a ones column for the softmax
     denominator).  k/v buffers carry 2 zero image-rows of padding on
     each side (padded neighbors give dots=0 -> exp(0)=1 in the
     denominator and v=0, matching the reference).
  C: per 256-px batch: banded transposed dots (4 chunks of 128 j; edge
     chunks only live on one 128-px half), exp on ACT, window/wrap mask
     multiply (GpSimd edge chunks / DVE middle chunks, dense
     slot-replicated masks), 3-chunk accumulating AV matmul per (head,
     pixel-half), normalize (reciprocal + broadcast TT mul), PE
     transpose, partial out-projection, fp16 DMA out.  Column-wrapped
     window positions are masked out and re-added to the denominator
     via n_pad.
"""

import numpy as np

B, HMAP, WMAP = 4, 64, 64
N = HMAP * WMAP          # 4096
DIM = 512
HEADS, HEAD_DIM = 8, 64
INNER = HEADS * HEAD_DIM  # 512
SCALE = HEAD_DIM ** -0.5
NB = N + 256             # padded k/v buffer pixels (2 zero rows each side)
NCHUNK = NB // 128       # 34
N_CORES = 8

_cache = {}


def _make_masks():
    """Window/wrap masks for the 4 chunks of a 256-px batch, plus n_pad.

    mask[c, j', p'] = 1 iff o = 128*c + j' - p' - 128 decomposes as
    64*di + dj with |di|,|dj| <= 2 and column p'%64 + dj stays in-image.
    n_pad[p] = number of column-invalid window positions for column p%64.
    """
    o = (128 * np.arange(4)[:, None, None] + np.arange(128)[None, :, None]
         - np.arange(256)[None, None, :] - 128)           # [4,128,256]
    di = np.round(o / 64.0).astype(np.int64)
    dj = o - 64 * di
    col = (np.arange(256) % 64)[None, None, :]
    ok = (np.abs(di) <= 2) & (np.abs(dj) <= 2) & (col + dj >= 0) & (col + dj < 64)
    masks = ok.astype(np.float16)
    colv = np.arange(64)
    npad_col = np.zeros(64, dtype=np.float32)
    for djv in range(-2, 3):
        npad_col += 5.0 * ((colv + djv < 0) | (colv + djv >= 64))
    n_pad = np.tile(npad_col, 2).reshape(128, 1).astype(np.float32)
    return masks, n_pad


def _build_nc():
    import concourse.bass as bass
    import concourse.tile as tile
    from concourse import mybir

    f32 = mybir.dt.float32
    f16 = mybir.dt.float16
    Exp = mybir.ActivationFunctionType.Exp

    from concourse import bacc
    nc = bacc.Bacc(None, target_bir_lowering=False)
    # xt/wqkvt/masks come pre-blocked from the host so every DMA descriptor
    # is a contiguous >=2KB per-partition run.
    xt_d = nc.dram_tensor("xt", [8, 128, 4, 512], f16, kind="ExternalInput")
    wqkvt_d = nc.dram_tensor("wqkvt", [128, 4, 768], f16, kind="ExternalInput")
    woutt_d = nc.dram_tensor("woutt", [256, DIM], f16, kind="ExternalInput")
    masks_d = nc.dram_tensor("masks", [128, 4, 256], f16, kind="ExternalInput")
    npad_d = nc.dram_tensor("npad", [128, 1], f32, kind="ExternalInput")
    ident_d = nc.dram_tensor("ident", [128, 128], f16, kind="ExternalInput")
    out_d = nc.dram_tensor("out", [N, DIM], f16, kind="ExternalOutput")

    with tile.TileContext(nc) as tc:
        from contextlib import ExitStack
        with ExitStack() as ctx:
            consts = ctx.enter_context(tc.tile_pool(name="consts", bufs=1))

            # per-kc split so the first projection chain can start as soon
            # as contraction chunk 0 of the weights + x block 0 lands
            # (instead of the full 1.3MB); remaining chunks stream in
            # behind them (emitted inside the blk==0 body).
            wqkvt = consts.tile([128, 4, 768], f16)
            nc.sync.dma_start(out=wqkvt[:, 0], in_=wqkvt_d[:, 0])

            woutt = consts.tile([128, 2, DIM], f16)
            masks = consts.tile([128, 4, 256], f16)
            # per-chunk masks replicated across the 4 head slots (dense
            # elementwise operand; broadcast APs run slower on DVE/GpSimd)
            masksf = consts.tile([128, 4, 4, 256], f16)
            npad = consts.tile([128, 1], f32)
            ident = consts.tile([128, 128], f16)

            # persistent activations
            qt = [consts.tile([128, N], f16, tag=f"qt{g}", name=f"qt{g}")
                  for g in range(2)]
            kt = [consts.tile([128, NB], f16, tag=f"kt{g}", name=f"kt{g}")
                  for g in range(2)]
            # v buffer: [p, chunk, 4 heads x (64 + ones col)]
            vsb = consts.tile([128, NCHUNK, 260], f16)

            for g in range(2):
                nc.vector.memset(kt[g][:, 0:128], 0.0)
                nc.vector.memset(kt[g][:, NB - 128:NB], 0.0)
            nc.vector.memset(vsb[:, 0, :], 0.0)
            nc.vector.memset(vsb[:, NCHUNK - 1, :], 0.0)
            ones_ap = vsb.rearrange("p c (h e) -> p c h e", h=4)[:, :, :, 64:65]
            nc.vector.memset(ones_ap, 1.0)

            # ---------------- Phase B: projections ----------------
            with ExitStack() as bctx:
                psb = bctx.enter_context(
                    tc.tile_pool(name="psum_b", bufs=2, space="PSUM"))
                xin = bctx.enter_context(tc.tile_pool(name="xin", bufs=2))
                for blk in range(8):
                    s0 = blk * 512
                    xtile = xin.tile([128, 4, 512], f16)
                    if blk == 0:
                        # startup-critical: land contraction chunk 0 first
                        nc.sync.dma_start(out=xtile[:, 0], in_=xt_d[0][:, 0])
                        for kc in range(1, 4):
                            nc.sync.dma_start(out=wqkvt[:, kc],
                                              in_=wqkvt_d[:, kc])
                            nc.sync.dma_start(out=xtile[:, kc],
                                              in_=xt_d[0][:, kc])
                    else:
                        nc.sync.dma_start(out=xtile, in_=xt_d[blk])
                    for m in range(4):  # q pair0, q pair1, k pair0, k pair1
                        ps = psb.tile([128, 512], f32, tag="psqk")
                        for kc in range(4):
                            nc.tensor.matmul(
                                ps,
                                wqkvt[:, kc, m * 128:(m + 1) * 128],
                                xtile[:, kc, :],
                                start=(kc == 0), stop=(kc == 3))
                        if m < 2:
                            nc.scalar.copy(qt[m][:, s0:s0 + 512], ps)
                        else:
                            nc.scalar.copy(
                                kt[m - 2][:, 128 + s0:128 + s0 + 512], ps)
                    for sub in range(4):
                        psv = psb.tile([128, 256], f32, tag="psv")
                        for kc in range(4):
                            nc.tensor.matmul(
                                psv,
                                xtile[:, kc, sub * 128:(sub + 1) * 128],
                                wqkvt[:, kc, 512:768],
                                start=(kc == 0), stop=(kc == 3))
                        ci = 1 + blk * 4 + sub
                        nc.vector.tensor_copy(
                            vsb[:, ci].rearrange(
                                "p (h e) -> p h e", h=4)[:, :, 0:64],
                            psv.rearrange("p (h e) -> p h e", h=4))

                    if blk == 0:
                        # phase-C constants: same HWDGE queue, issued after
                        # the startup-critical wqkvt + first xtile
                        nc.sync.dma_start(
                            out=woutt,
                            in_=woutt_d.rearrange("(c p) m -> p c m", p=128))
                        nc.sync.dma_start(out=masks, in_=masks_d[:, :, :])
                        nc.sync.dma_start(out=npad, in_=npad_d[:, :])
                        nc.sync.dma_start(out=ident, in_=ident_d[:, :])
                        for c4 in range(4):
                            lo, hi = (0, 128) if c4 == 0 else \
                                     (128, 256) if c4 == 3 else (0, 256)
                            nc.gpsimd.tensor_copy(
                                masksf[:, c4, :, lo:hi],
                                masks[:, c4, lo:hi].unsqueeze(1)
                                .to_broadcast([128, 4, hi - lo]))

            # ---------------- Phase C: attention + projection ----------------
            with ExitStack() as cctx:
                pspw = cctx.enter_context(
                    tc.tile_pool(name="psum_pw", bufs=2, space="PSUM"))
                pso = cctx.enter_context(
                    tc.tile_pool(name="psum_o", bufs=1, space="PSUM"))
                pst = cctx.enter_context(
                    tc.tile_pool(name="psum_t", bufs=1, space="PSUM"))
                pspj = cctx.enter_context(
                    tc.tile_pool(name="psum_pj", bufs=1, space="PSUM"))
                epool = cctx.enter_context(tc.tile_pool(name="em", bufs=2))
                opool = cctx.enter_context(tc.tile_pool(name="oc", bufs=2))
                dpool = cctx.enter_context(tc.tile_pool(name="den", bufs=2))

                def stage_a(si):
                    """dots + exp + mask for batch si -> em tile"""
                    s = si * 256
                    # em: [j, chunk, slot, p']; edge-chunk dead halves are
                    # never written nor read (po skips c=3 for ph=0 and
                    # c=0 for ph=1).
                    em = epool.tile([128, 4, 4, 256], f16, tag="em",
                                    name=f"em{si}")
                    for c in range(4):
                        lo, hi = (0, 128) if c == 0 else \
                                 (128, 256) if c == 3 else (0, 256)
                        # slot order (hs, g): concurrent row-group matmuls
                        # (hs=0 vs hs=1) must land in different PSUM banks.
                        pw = pspw.tile([128, 2, 2, 256], f32)
                        for g in range(2):
                            for hs in range(2):
                                lo_p, hi_p = hs * 64, (hs + 1) * 64
                                nc.tensor.matmul(
                                    pw[:, hs, g, lo:hi],
                                    kt[g][lo_p:hi_p,
                                          s + 128 * c:s + 128 * c + 128],
                                    qt[g][lo_p:hi_p, s + lo:s + hi],
                                    start=True, stop=True)
                        er = epool.tile([128, 4, 256], f16, tag="er")
                        nc.scalar.activation(
                            out=er[:, :, lo:hi],
                            in_=pw.rearrange(
                                "p a b f -> p (a b) f")[:, :, lo:hi],
                            func=Exp, scale=SCALE)
                        nc.vector.tensor_mul(
                            em[:, c, :, lo:hi], er[:, :, lo:hi],
                            masksf[:, c, :, lo:hi])
                    return em

                def stage_b(si, em):
                    """AV + normalize + transpose + projection + out"""
                    s = si * 256
                    po = [pso.tile([128, 4, 65], f32, tag=f"po{ph}",
                                   name=f"po{ph}") for ph in range(2)]
                    for gh in range(4):
                        slot = 2 * (gh % 2) + gh // 2
                        for ph in range(2):
                            cs = (0, 1, 2) if ph == 0 else (1, 2, 3)
                            for i, c in enumerate(cs):
                                nc.tensor.matmul(
                                    po[ph][:, gh, 0:65],
                                    em[:, c, slot, ph * 128:(ph + 1) * 128],
                                    vsb[:, 2 * si + c, 65 * gh:65 * gh + 65],
                                    start=(i == 0), stop=(i == 2))
                    den = dpool.tile([128, 2, 4], f32, tag="den")
                    for ph in range(2):
                        nc.vector.tensor_add(
                            den[:, ph, :].unsqueeze(2),
                            po[ph][:, :, 64:65],
                            npad.unsqueeze(2).to_broadcast([128, 4, 1]))
                    rec = dpool.tile([128, 2, 4], f32, tag="rec")
                    nc.vector.reciprocal(rec, den)

                    ob = opool.tile([128, 2, DIM], f16, tag="ob")
                    for ph in range(2):
                        opix = opool.tile([128, 256], f16, tag="opix")
                        nc.vector.tensor_mul(
                            opix.rearrange("p (g e) -> p g e", g=4),
                            po[ph][:, :, 0:64],
                            rec[:, ph, :].unsqueeze(2).to_broadcast(
                                [128, 4, 64]))
                        otb = opool.tile([128, 2, 128], f16, tag="otb")
                        for i in range(2):
                            pt = pst.tile([128, 128], f16)
                            nc.tensor.transpose(
                                pt, opix[:, i * 128:(i + 1) * 128], ident)
                            nc.vector.tensor_copy(otb[:, i], pt)
                        pj = pspj.tile([128, DIM], f32)
                        for i in range(2):
                            nc.tensor.matmul(
                                pj, otb[:, i], woutt[:, i],
                                start=(i == 0), stop=(i == 1))
                        if ph == 0:
                            nc.vector.tensor_copy(ob[:, 0], pj)
                        else:
                            nc.scalar.copy(ob[:, 1], pj)
                    # one DMA for both 128-px halves (fewer DMA instrs +
                    # semaphore waits)
                    nc.sync.dma_start(
                        out=out_d[s:s + 256, :].rearrange(
                            "(h p) m -> p h m", p=128),
                        in_=ob)

                # straight emission; the Tile scheduler already overlaps
                # consecutive batches (an explicit software pipeline of
                # stage A ahead of stage B measured slightly worse)
                for si in range(16):
                    stage_b(si, stage_a(si))

    nc.finalize()
    return nc


def _prepare_core_inputs(x, w_qkv, w_out, b_out):
    masks, n_pad = _make_masks()
    masks_p = np.ascontiguousarray(masks.transpose(1, 0, 2))  # [128, 4, 256]
    ident = np.eye(128, dtype=np.float16)
    per_core = []
    for ci in range(N_CORES):
        b, hg = ci // 2, ci % 2
        q_rows = w_qkv[256 * hg:256 * hg + 256]
        k_rows = w_qkv[INNER + 256 * hg:INNER + 256 * hg + 256]
        v_rows = w_qkv[2 * INNER + 256 * hg:2 * INNER + 256 * hg + 256]
        w_slice = np.concatenate([q_rows, k_rows, v_rows], axis=0)  # [768, 512]
        xt = x[b].T.astype(np.float16)                       # [512, 4096]
        # blocked: [blk, p, c, n-slice] so each per-partition DMA run is 4KB
        xtb = np.ascontiguousarray(
            xt.reshape(4, 128, 8, 512).transpose(2, 1, 0, 3))
        wq = np.ascontiguousarray(
            w_slice.T.astype(np.float16).reshape(4, 128, 768).transpose(1, 0, 2))
        per_core.append({
            "xt": xtb,
            "wqkvt": wq,
            "woutt": np.ascontiguousarray(
                w_out[:, 256 * hg:256 * hg + 256].T).astype(np.float16),
            "masks": masks_p,
            "npad": n_pad,
            "ident": ident,
        })
    return per_core


def kernel(x, w_qkv, w_out, b_out, h, w):
    assert int(h) == HMAP and int(w) == WMAP
    x = np.asarray(x, dtype=np.float32)
    w_qkv = np.asarray(w_qkv, dtype=np.float32)
    w_out = np.asarray(w_out, dtype=np.float32)
    b_out = np.asarray(b_out, dtype=np.float32)

    if "nc" not in _cache:
        _cache["nc"] = _build_nc()
    nc = _cache["nc"]

    from concourse.bass_utils import run_bass_kernel_spmd
    in_maps = _prepare_core_inputs(x, w_qkv, w_out, b_out)
    res = run_bass_kernel_spmd(nc, in_maps, core_ids=list(range(N_CORES)))
    out = np.zeros((B, N, DIM), dtype=np.float32)
    for b in range(B):
        out[b] = (res.results[2 * b]["out"].astype(np.float32)
                  + res.results[2 * b + 1]["out"].astype(np.float32)
                  + b_out[None, :])
    return out


# revision 30
# speedup vs baseline: 1.0328x; 1.0328x over previous
"""Trainium2 Bass kernel for nn_LocalAttention (5x5 local window attention).

Contract: kernel(**inputs) takes the FULL inputs from setup_inputs() and
returns the FULL output.  Internally shards across 8 NeuronCores as
(batch b in 0..3) x (head-group hg in 0..1, 4 heads each).  Each core
computes a partial output projection; the host sums the two partials per
batch and adds b_out.

Two phases (no barrier; Tile dependency tracking orders them):
  B: qkv projection of 512-px blocks into persistent qT/kT (d-major,
     fp16) and v (pixel-major fp16 with a ones column for the softmax
     denominator).  k/v buffers carry 2 zero image-rows of padding on
     each side (padded neighbors give dots=0 -> exp(0)=1 in the
     denominator and v=0, matching the reference).
  C: per 256-px batch: banded transposed dots (4 chunks of 128 j; edge
     chunks only live on one 128-px half), exp on ACT, window/wrap mask
     multiply (GpSimd edge chunks / DVE middle chunks, dense
     slot-replicated masks), 3-chunk accumulating AV matmul per (head,
     pixel-half), normalize (reciprocal + broadcast TT mul), PE
     transpose, partial out-projection, fp16 DMA out.  Column-wrapped
     window positions are masked out and re-added to the denominator
     via n_pad.
"""

import numpy as np

B, HMAP, WMAP = 4, 64, 64
N = HMAP * WMAP          # 4096
DIM = 512
HEADS, HEAD_DIM = 8, 64
INNER = HEADS * HEAD_DIM  # 512
SCALE = HEAD_DIM ** -0.5
NB = N + 256             # padded k/v buffer pixels (2 zero rows each side)
NCHUNK = NB // 128       # 34
N_CORES = 8

_cache = {}


def _make_masks():
    """Window/wrap masks for the 4 chunks of a 256-px batch, plus n_pad.

    mask[c, j', p'] = 1 iff o = 128*c + j' - p' - 128 decomposes as
    64*di + dj with |di|,|dj| <= 2 and column p'%64 + dj stays in-image.
    n_pad[p] = number of column-invalid window positions for column p%64.
    """
    o = (128 * np.arange(4)[:, None, None] + np.arange(128)[None, :, None]
         - np.arange(256)[None, None, :] - 128)           # [4,128,256]
    di = np.round(o / 64.0).astype(np.int64)
    dj = o - 64 * di
    col = (np.arange(256) % 64)[None, None, :]
    ok = (np.abs(di) <= 2) & (np.abs(dj) <= 2) & (col + dj >= 0) & (col + dj < 64)
    masks = ok.astype(np.float16)
    colv = np.arange(64)
    npad_col = np.zeros(64, dtype=np.float32)
    for djv in range(-2, 3):
        npad_col += 5.0 * ((colv + djv < 0) | (colv + djv >= 64))
    n_pad = np.tile(npad_col, 2).reshape(128, 1).astype(np.float32)
    return masks, n_pad


def _build_nc():
    import concourse.bass as bass
    import concourse.tile as tile
    from concourse import mybir

    f32 = mybir.dt.float32
    f16 = mybir.dt.float16
    Exp = mybir.ActivationFunctionType.Exp

    from concourse import bacc
    nc = bacc.Bacc(None, target_bir_lowering=False)
    # xt/wqkvt/masks come pre-blocked from the host so every DMA descriptor
    # is a contiguous >=2KB per-partition run.
    xt_d = nc.dram_tensor("xt", [8, 128, 4, 512], f16, kind="ExternalInput")
    wqkvt_d = nc.dram_tensor("wqkvt", [128, 4, 768], f16, kind="ExternalInput")
    woutt_d = nc.dram_tensor("woutt", [256, DIM], f16, kind="ExternalInput")
    masks_d = nc.dram_tensor("masks", [128, 4, 256], f16, kind="ExternalInput")
    npad_d = nc.dram_tensor("npad", [128, 1], f32, kind="ExternalInput")
    ident_d = nc.dram_tensor("ident", [128, 128], f16, kind="ExternalInput")
    out_d = nc.dram_tensor("out", [N, DIM], f16, kind="ExternalOutput")

    with tile.TileContext(nc) as tc:
        from contextlib import ExitStack
        with ExitStack() as ctx:
            consts = ctx.enter_context(tc.tile_pool(name="consts", bufs=1))

            wqkvt = consts.tile([128, 4, 768], f16)
            nc.sync.dma_start(out=wqkvt[:, 0], in_=wqkvt_d[:, 0])

            woutt = consts.tile([128, 2, DIM], f16)
            masks = consts.tile([128, 4, 256], f16)
            # per-chunk masks replicated across the 4 head slots (dense
            # elementwise operand; broadcast APs run slower on DVE/GpSimd)
            masksf = consts.tile([128, 4, 4, 256], f16)
            npad = consts.tile([128, 1], f32)
            ident = consts.tile([128, 128], f16)

            # persistent activations
            qt = [consts.tile([128, N], f16, tag=f"qt{g}", name=f"qt{g}")
                  for g in range(2)]
            kt = [consts.tile([128, NB], f16, tag=f"kt{g}", name=f"kt{g}")
                  for g in range(2)]
            # v buffer: [p, chunk, 4 heads x (64 + ones col)]
            vsb = consts.tile([128, NCHUNK, 260], f16)

            for g in range(2):
                nc.vector.memset(kt[g][:, 0:128], 0.0)
                nc.vector.memset(kt[g][:, NB - 128:NB], 0.0)
            nc.vector.memset(vsb[:, 0, :], 0.0)
            nc.vector.memset(vsb[:, NCHUNK - 1, :], 0.0)
            ones_ap = vsb.rearrange("p c (h e) -> p c h e", h=4)[:, :, :, 64:65]
            nc.vector.memset(ones_ap, 1.0)

            # ---------------- Phase B: projections ----------------
            with ExitStack() as bctx:
                psb = bctx.enter_context(
                    tc.tile_pool(name="psum_b", bufs=2, space="PSUM"))
                xin = bctx.enter_context(tc.tile_pool(name="xin", bufs=2))
                for blk in range(8):
                    s0 = blk * 512
                    xtile = xin.tile([128, 4, 512], f16)
                    if blk == 0:
                        # startup-critical: land contraction chunk 0 first
                        nc.sync.dma_start(out=xtile[:, 0], in_=xt_d[0][:, 0])
                        for kc in range(1, 4):
                            nc.sync.dma_start(out=wqkvt[:, kc],
                                              in_=wqkvt_d[:, kc])
                            nc.sync.dma_start(out=xtile[:, kc],
                                              in_=xt_d[0][:, kc])
                    else:
                        nc.sync.dma_start(out=xtile, in_=xt_d[blk])
                    for m in range(4):  # q pair0, q pair1, k pair0, k pair1
                        ps = psb.tile([128, 512], f32, tag="psqk")
                        for kc in range(4):
                            nc.tensor.matmul(
                                ps,
                                wqkvt[:, kc, m * 128:(m + 1) * 128],
                                xtile[:, kc, :],
                                start=(kc == 0), stop=(kc == 3))
                        if m < 2:
                            nc.scalar.copy(qt[m][:, s0:s0 + 512], ps)
                        else:
                            nc.scalar.copy(
                                kt[m - 2][:, 128 + s0:128 + s0 + 512], ps)
                    for sub in range(4):
                        psv = psb.tile([128, 256], f32, tag="psv")
                        for kc in range(4):
                            nc.tensor.matmul(
                                psv,
                                xtile[:, kc, sub * 128:(sub + 1) * 128],
                                wqkvt[:, kc, 512:768],
                                start=(kc == 0), stop=(kc == 3))
                        ci = 1 + blk * 4 + sub
                        nc.vector.tensor_copy(
                            vsb[:, ci].rearrange(
                                "p (h e) -> p h e", h=4)[:, :, 0:64],
                            psv.rearrange("p (h e) -> p h e", h=4))

                    if blk == 0:
                        # phase-C constants: same HWDGE queue, issued after
                        # the startup-critical wqkvt + first xtile
                        nc.sync.dma_start(
                            out=woutt,
                            in_=woutt_d.rearrange("(c p) m -> p c m", p=128))
                        nc.sync.dma_start(out=masks, in_=masks_d[:, :, :])
                        nc.sync.dma_start(out=npad, in_=npad_d[:, :])
                        nc.sync.dma_start(out=ident, in_=ident_d[:, :])
                        for c4 in range(4):
                            lo, hi = (0, 128) if c4 == 0 else \
                                     (128, 256) if c4 == 3 else (0, 256)
                            nc.gpsimd.tensor_copy(
                                masksf[:, c4, :, lo:hi],
                                masks[:, c4, lo:hi].unsqueeze(1)
                                .to_broadcast([128, 4, hi - lo]))

            # ---------------- Phase C: attention + projection ----------------
            with ExitStack() as cctx:
                pspw = cctx.enter_context(
                    tc.tile_pool(name="psum_pw", bufs=2, space="PSUM"))
                pso = cctx.enter_context(
                    tc.tile_pool(name="psum_o", bufs=1, space="PSUM"))
                pst = cctx.enter_context(
                    tc.tile_pool(name="psum_t", bufs=1, space="PSUM"))
                pspj = cctx.enter_context(
                    tc.tile_pool(name="psum_pj", bufs=1, space="PSUM"))
                epool = cctx.enter_context(tc.tile_pool(name="em", bufs=2))
                opool = cctx.enter_context(tc.tile_pool(name="oc", bufs=2))
                dpool = cctx.enter_context(tc.tile_pool(name="den", bufs=2))

                def stage_a(si):
                    """dots + exp + mask for batch si -> em tile"""
                    s = si * 256
                    # em: [j, chunk, slot, p']; edge-chunk dead halves are
                    # never written nor read (po skips c=3 for ph=0 and
                    # c=0 for ph=1).
                    em = epool.tile([128, 4, 4, 256], f16, tag="em",
                                    name=f"em{si}")
                    for c in range(4):
                        lo, hi = (0, 128) if c == 0 else \
                                 (128, 256) if c == 3 else (0, 256)
                        # slot order (hs, g): concurrent row-group matmuls
                        # (hs=0 vs hs=1) must land in different PSUM banks.
                        pw = pspw.tile([128, 2, 2, 256], f32)
                        for g in range(2):
                            for hs in range(2):
                                lo_p, hi_p = hs * 64, (hs + 1) * 64
                                nc.tensor.matmul(
                                    pw[:, hs, g, lo:hi],
                                    kt[g][lo_p:hi_p,
                                          s + 128 * c:s + 128 * c + 128],
                                    qt[g][lo_p:hi_p, s + lo:s + hi],
                                    start=True, stop=True)
                        er = epool.tile([128, 4, 256], f16, tag="er")
                        nc.scalar.activation(
                            out=er[:, :, lo:hi],
                            in_=pw.rearrange(
                                "p a b f -> p (a b) f")[:, :, lo:hi],
                            func=Exp, scale=SCALE)
                        nc.vector.tensor_mul(
                            em[:, c, :, lo:hi], er[:, :, lo:hi],
                            masksf[:, c, :, lo:hi])
                    return em

                def stage_b(si, em):
                    """AV + normalize + transpose + projection + out"""
                    s = si * 256
                    po = [pso.tile([128, 4, 65], f32, tag=f"po{ph}",
                                   name=f"po{ph}") for ph in range(2)]
                    for gh in range(4):
                        slot = 2 * (gh % 2) + gh // 2
                        for ph in range(2):
                            cs = (0, 1, 2) if ph == 0 else (1, 2, 3)
                            for i, c in enumerate(cs):
                                nc.tensor.matmul(
                                    po[ph][:, gh, 0:65],
                                    em[:, c, slot, ph * 128:(ph + 1) * 128],
                                    vsb[:, 2 * si + c, 65 * gh:65 * gh + 65],
                                    start=(i == 0), stop=(i == 2))
                    den = dpool.tile([128, 2, 4], f32, tag="den")
                    for ph in range(2):
                        nc.vector.tensor_add(
                            den[:, ph, :].unsqueeze(2),
                            po[ph][:, :, 64:65],
                            npad.unsqueeze(2).to_broadcast([128, 4, 1]))
                    rec = dpool.tile([128, 2, 4], f32, tag="rec")
                    nc.vector.reciprocal(rec, den)

                    ob = opool.tile([128, 2, DIM], f16, tag="ob")
                    for ph in range(2):
                        opix = opool.tile([128, 256], f16, tag="opix")
                        nc.vector.tensor_mul(
                            opix.rearrange("p (g e) -> p g e", g=4),
                            po[ph][:, :, 0:64],
                            rec[:, ph, :].unsqueeze(2).to_broadcast(
                                [128, 4, 64]))
                        otb = opool.tile([128, 2, 128], f16, tag="otb")
                        for i in range(2):
                            pt = pst.tile([128, 128], f16)
                            nc.tensor.transpose(
                                pt, opix[:, i * 128:(i + 1) * 128], ident)
                            nc.vector.tensor_copy(otb[:, i], pt)
                        pj = pspj.tile([128, DIM], f32)
                        for i in range(2):
                            nc.tensor.matmul(
                                pj, otb[:, i], woutt[:, i],
                                start=(i == 0), stop=(i == 1))
                        if ph == 0:
                            nc.vector.tensor_copy(ob[:, 0], pj)
                        else:
                            nc.scalar.copy(ob[:, 1], pj)
                    # one DMA for both 128-px halves (fewer DMA instrs +
                    # semaphore waits)
                    nc.sync.dma_start(
                        out=out_d[s:s + 256, :].rearrange(
                            "(h p) m -> p h m", p=128),
                        in_=ob)

                # straight emission; the Tile scheduler already overlaps
                # consecutive batches (an explicit software pipeline of
                # stage A ahead of stage B measured slightly worse)
                for si in range(16):
                    stage_b(si, stage_a(si))

    nc.finalize()
    return nc


def _prepare_core_inputs(x, w_qkv, w_out, b_out):
    masks, n_pad = _make_masks()
    masks_p = np.ascontiguousarray(masks.transpose(1, 0, 2))  # [128, 4, 256]
    ident = np.eye(128, dtype=np.float16)
    per_core = []
    for ci in range(N_CORES):
        b, hg = ci // 2, ci % 2
        q_rows = w_qkv[256 * hg:256 * hg + 256]
        k_rows = w_qkv[INNER + 256 * hg:INNER + 256 * hg + 256]
        v_rows = w_qkv[2 * INNER + 256 * hg:2 * INNER + 256 * hg + 256]
        w_slice = np.concatenate([q_rows, k_rows, v_rows], axis=0)  # [768, 512]
        xt = x[b].T.astype(np.float16)                       # [512, 4096]
        # blocked: [blk, p, c, n-slice] so each per-partition DMA run is 4KB
        xtb = np.ascontiguousarray(
            xt.reshape(4, 128, 8, 512).transpose(2, 1, 0, 3))
        wq = np.ascontiguousarray(
            w_slice.T.astype(np.float16).reshape(4, 128, 768).transpose(1, 0, 2))
        per_core.append({
            "xt": xtb,
            "wqkvt": wq,
            "woutt": np.ascontiguousarray(
                w_out[:, 256 * hg:256 * hg + 256].T).astype(np.float16),
            "masks": masks_p,
            "npad": n_pad,
            "ident": ident,
        })
    return per_core


def kernel(x, w_qkv, w_out, b_out, h, w):
    assert int(h) == HMAP and int(w) == WMAP
    x = np.asarray(x, dtype=np.float32)
    w_qkv = np.asarray(w_qkv, dtype=np.float32)
    w_out = np.asarray(w_out, dtype=np.float32)
    b_out = np.asarray(b_out, dtype=np.float32)

    if "nc" not in _cache:
        _cache["nc"] = _build_nc()
    nc = _cache["nc"]

    from concourse.bass_utils import run_bass_kernel_spmd
    in_maps = _prepare_core_inputs(x, w_qkv, w_out, b_out)
    res = run_bass_kernel_spmd(nc, in_maps, core_ids=list(range(N_CORES)))
    out = np.zeros((B, N, DIM), dtype=np.float32)
    for b in range(B):
        out[b] = (res.results[2 * b]["out"].astype(np.float32)
                  + res.results[2 * b + 1]["out"].astype(np.float32)
                  + b_out[None, :])
    return out
